# revision 1
# baseline (speedup 1.0000x reference)
"""BipartiteGCN message-passing kernel for 8 TRN2 NeuronCores.

Math:  out = D_c^{-1/2} A^T D_r^{-1/2} (x @ W) + b
where A[s, d] = multiplicity of edge (gene s, drug d), s, d in [0, 4000).

Strategy (dst-window sharding, no output all-reduce):
  - Core c owns drug (dst) window [512c, 512c+512).  Edges are sharded to
    cores by dst window and sorted by src gene (host-side layout only; all
    arithmetic happens on device).
  - Each core builds its dense count stripe A_c [4096 genes x 512 drugs]
    directly in SBUF with one-hot x one-hot PE matmuls: for each 128-edge
    chunk, lhsT[e, g] = (src_e == g), rhs[e, d] = (dst_e == d) (fp16
    one-hots built by DVE compare-vs-iota), accumulated per 128-gene window
    in fp32 PSUM.  No gather/scatter DMA at all.
  - xW is computed row-sharded over genes (512 rows/core) and all-gathered.
  - row_deg = free-axis rowsums of A_c (partial -> 16KB AllReduce);
    col_deg = ones^T @ A_c on the PE (local).  f = rsqrt-masked row_deg,
    g = rsqrt-masked col_deg, all on device.
  - out_c = g * ((f*A_c)^T @ xWf) + bias ; host concatenates the stripes.
"""

import sys

if "/opt/trn_rl_repo" not in sys.path:
    sys.path.insert(0, "/opt/trn_rl_repo")

import numpy as np

import concourse.bass as bass  # noqa: F401
import concourse.mybir as mybir
from concourse import bacc, tile

CORES = 8
DWIN = 512              # dst (drug) window per core
ND = 4000               # number of drugs
GD = 4096               # padded gene dim (src < 4000)
IC = 1024
OC = 512
ST = GD // 128          # 32 gene windows / tiles
WCH = 10                # 128-edge chunks per gene window (max 1172 edges)
NCH = ST * WCH          # 320 chunks per core
NSLOT = NCH * 128       # 40960 edge slots per core

F32 = mybir.dt.float32
F16 = mybir.dt.float16


def build_nc(debug_outputs=False):
    nc = bacc.Bacc(
        None,
        target_bir_lowering=False,
        debug=False,
        num_devices=CORES,
    )

    xT = nc.dram_tensor("xT", [IC, DWIN], F32, kind="ExternalInput")
    w = nc.dram_tensor("w", [IC, OC], F32, kind="ExternalInput")
    brep = nc.dram_tensor("brep", [128, OC], F32, kind="ExternalInput")
    i128 = nc.dram_tensor("i128", [128, 128], F16, kind="ExternalInput")
    i512 = nc.dram_tensor("i512", [128, OC], F16, kind="ExternalInput")
    sloc = nc.dram_tensor("sloc", [128, NCH], F32, kind="ExternalInput")
    dloc = nc.dram_tensor("dloc", [128, NCH], F32, kind="ExternalInput")
    out = nc.dram_tensor("out", [DWIN, OC], F32, kind="ExternalOutput")

    xw0l = nc.dram_tensor("xw0l", [DWIN, OC], F32)         # local xW stripe
    xw0f = nc.dram_tensor("xw0f", [GD, OC], F32, addr_space="Shared")
    rdl = nc.dram_tensor("rdl", [128, ST], F32)            # rowdeg partial
    rds = nc.dram_tensor("rds", [128, ST], F32, addr_space="Shared")

    Adbg = xwdbg = None
    if debug_outputs:
        Adbg = nc.dram_tensor("Adbg", [GD, OC], F32, kind="ExternalOutput")
        xwdbg = nc.dram_tensor("xwdbg", [GD, OC], F32, kind="ExternalOutput")

    with tile.TileContext(nc) as tc:
        with (
            tc.tile_pool(name="const", bufs=1) as cpool,
            tc.tile_pool(name="work", bufs=2) as wpool,
            tc.tile_pool(name="apool", bufs=ST) as apool,
            tc.tile_pool(name="psum", bufs=4, space="PSUM") as ppool,
        ):
            # constants
            ones_sb = cpool.tile([128, 1], F32)
            nc.vector.memset(ones_sb[:], 1.0)
            i128_sb = cpool.tile([128, 128], F16)
            nc.sync.dma_start(i128_sb[:], i128[:])
            i512_sb = cpool.tile([128, OC], F16)
            nc.sync.dma_start(i512_sb[:], i512[:])
            bias_sb = cpool.tile([128, OC], F32)
            nc.sync.dma_start(bias_sb[:], brep[:])
            sloc_sb = cpool.tile([128, NCH], F32)
            nc.sync.dma_start(sloc_sb[:], sloc[:])
            dloc_sb = cpool.tile([128, NCH], F32)
            nc.sync.dma_start(dloc_sb[:], dloc[:])

            # phase B: xw0 = x_shard @ W  (genes 512c..512c+512)
            pb = [ppool.tile([128, OC], F32, tag="acc", name=f"pb{i}") for i in range(4)]
            for kt in range(8):
                xt_t = wpool.tile([128, DWIN], F32, tag="xT", name=f"xt{kt}")
                w_t = wpool.tile([128, OC], F32, tag="w", name=f"w{kt}")
                nc.sync.dma_start(xt_t[:], xT[kt * 128:(kt + 1) * 128, :])
                nc.sync.dma_start(w_t[:], w[kt * 128:(kt + 1) * 128, :])
                for mt in range(4):
                    nc.tensor.matmul(
                        pb[mt][:],
                        xt_t[:, mt * 128:(mt + 1) * 128],
                        w_t[:],
                        start=(kt == 0),
                        stop=(kt == 7),
                    )
            for mt in range(4):
                o = wpool.tile([128, OC], F32, tag="xw0sb", name=f"xw0sb{mt}")
                nc.vector.tensor_copy(o[:], pb[mt][:])
                nc.sync.dma_start(xw0l[mt * 128:(mt + 1) * 128, :], o[:])

            # phase C: all-gather xW  (rank r -> rows 512r..512r+512)
            nc.gpsimd.collective_compute(
                "AllGather",
                mybir.AluOpType.bypass,
                replica_groups=[list(range(CORES))],
                ins=[xw0l[:].opt()],
                outs=[xw0f[:].opt()],
            )

            # phase D: build the A stripe in SBUF, one 128-gene window at a
            # time, as sums of one-hot outer products on the PE.  Also emits
            # the row-degree partials (free-axis rowsums).
            a_sb = []
            rd_sb = cpool.tile([128, ST], F32)
            for t in range(ST):
                pa = ppool.tile([128, OC], F32, tag="bld", bufs=2, name=f"pa{t}")
                for i in range(WCH):
                    c = t * WCH + i
                    loh = wpool.tile([128, 128], F16, tag="loh", bufs=3,
                                     name=f"loh{c}")
                    roh = wpool.tile([128, OC], F16, tag="roh", bufs=3,
                                     name=f"roh{c}")
                    nc.vector.tensor_scalar(
                        out=loh[:], in0=i128_sb[:],
                        scalar1=sloc_sb[:, c:c + 1], scalar2=None,
                        op0=mybir.AluOpType.is_equal,
                    )
                    nc.vector.tensor_scalar(
                        out=roh[:], in0=i512_sb[:],
                        scalar1=dloc_sb[:, c:c + 1], scalar2=None,
                        op0=mybir.AluOpType.is_equal,
                    )
                    nc.tensor.matmul(
                        pa[:], loh[:], roh[:],
                        start=(i == 0), stop=(i == WCH - 1),
                    )
                a_t = apool.tile([128, OC], F32, tag="A", name=f"a{t}")
                nc.scalar.copy(a_t[:], pa[:])
                a_sb.append(a_t)
                if debug_outputs:
                    nc.sync.dma_start(Adbg[t * 128:(t + 1) * 128, :], a_t[:])
                nc.vector.reduce_sum(
                    rd_sb[:, t:t + 1], a_t[:], axis=mybir.AxisListType.X
                )

            # col_deg = ones^T @ A  ([1, 512] psum accumulated over windows)
            pcd = ppool.tile([1, OC], F32, tag="cd", bufs=1)
            for t in range(ST):
                nc.tensor.matmul(
                    pcd[:], ones_sb[:], a_sb[t][:],
                    start=(t == 0), stop=(t == ST - 1),
                )
            cd_row = cpool.tile([1, OC], F32)
            nc.vector.tensor_copy(cd_row[:], pcd[:])
            # redistribute [1, 512] -> [128, 4]: column dt holds drugs
            # dt*128 + p on partition p (matches phase G's per-partition g)
            cd_sb = cpool.tile([128, 4], F32)
            for kq in range(4):
                nc.sync.dma_start(
                    cd_sb[:, kq:kq + 1], cd_row[0:1, kq * 128:(kq + 1) * 128]
                )

            # row_deg all-reduce and f = (deg>0)/sqrt(max(deg,1))
            nc.sync.dma_start(rdl[:], rd_sb[:])
            nc.gpsimd.collective_compute(
                "AllReduce",
                mybir.AluOpType.add,
                replica_groups=[list(range(CORES))],
                ins=[rdl[:].opt()],
                outs=[rds[:].opt()],
            )
            deg_sb = cpool.tile([128, ST], F32)
            nc.sync.dma_start(deg_sb[:], rds[:])
            t1 = cpool.tile([128, ST], F32)
            nc.vector.tensor_scalar(
                out=t1[:], in0=deg_sb[:], scalar1=1.0, scalar2=None,
                op0=mybir.AluOpType.max,
            )
            nc.scalar.sqrt(t1[:], t1[:])
            nc.vector.reciprocal(t1[:], t1[:])
            fmask = cpool.tile([128, ST], F32)
            nc.vector.tensor_scalar(
                out=fmask[:], in0=deg_sb[:], scalar1=0.5, scalar2=None,
                op0=mybir.AluOpType.is_gt,
            )
            f_sb = cpool.tile([128, ST], F32)
            nc.vector.tensor_tensor(
                out=f_sb[:], in0=t1[:], in1=fmask[:], op=mybir.AluOpType.mult
            )

            # g = (coldeg>0)/sqrt(max(coldeg,1))   [128, 4]
            g1 = cpool.tile([128, 4], F32)
            nc.vector.tensor_scalar(
                out=g1[:], in0=cd_sb[:], scalar1=1.0, scalar2=None,
                op0=mybir.AluOpType.max,
            )
            nc.scalar.sqrt(g1[:], g1[:])
            nc.vector.reciprocal(g1[:], g1[:])
            gmask = cpool.tile([128, 4], F32)
            nc.vector.tensor_scalar(
                out=gmask[:], in0=cd_sb[:], scalar1=0.5, scalar2=None,
                op0=mybir.AluOpType.is_gt,
            )
            g_sb = cpool.tile([128, 4], F32)
            nc.vector.tensor_tensor(
                out=g_sb[:], in0=g1[:], in1=gmask[:], op=mybir.AluOpType.mult
            )

            # phase F: out = (f*A)^T @ xw0f  accumulated over gene windows
            po = [ppool.tile([128, OC], F32, tag="acc", name=f"po{i}") for i in range(4)]
            for t in range(ST):
                nc.vector.tensor_scalar(
                    out=a_sb[t][:], in0=a_sb[t][:],
                    scalar1=f_sb[:, t:t + 1], scalar2=None,
                    op0=mybir.AluOpType.mult,
                )
                xf_t = wpool.tile([128, OC], F32, tag="xwf", bufs=3, name=f"xf{t}")
                nc.sync.dma_start(xf_t[:], xw0f[t * 128:(t + 1) * 128, :])
                if debug_outputs:
                    nc.sync.dma_start(xwdbg[t * 128:(t + 1) * 128, :], xf_t[:])
                for dt in range(4):
                    nc.tensor.matmul(
                        po[dt][:],
                        a_sb[t][:, dt * 128:(dt + 1) * 128],
                        xf_t[:],
                        start=(t == 0),
                        stop=(t == ST - 1),
                    )

            # phase G: scale by g, add bias, store
            for dt in range(4):
                og = wpool.tile([128, OC], F32, tag="og", name=f"og{dt}")
                nc.vector.tensor_scalar(
                    out=og[:], in0=po[dt][:],
                    scalar1=g_sb[:, dt:dt + 1], scalar2=None,
                    op0=mybir.AluOpType.mult,
                )
                nc.vector.tensor_tensor(
                    out=og[:], in0=og[:], in1=bias_sb[:], op=mybir.AluOpType.add
                )
                nc.sync.dma_start(out[dt * 128:(dt + 1) * 128, :], og[:])

    nc.finalize()
    return nc


def make_in_maps(x, weight, bias, edge_index):
    """Host-side sharding/layout only: no arithmetic on tensor values."""
    x = np.asarray(x, dtype=np.float32)
    weight = np.ascontiguousarray(np.asarray(weight, dtype=np.float32))
    bias = np.asarray(bias, dtype=np.float32)
    ei = np.asarray(edge_index)
    s_all = ei[0].astype(np.int64)
    d_all = ei[1].astype(np.int64)
    assert s_all.min() >= 0 and s_all.max() < ND, "src ids out of supported range"
    assert d_all.min() >= 0 and d_all.max() < ND, "dst ids out of supported range"

    brep = np.ascontiguousarray(np.tile(bias[None, :], (128, 1)).astype(np.float32))
    i128 = np.ascontiguousarray(
        np.tile(np.arange(128, dtype=np.float16)[None, :], (128, 1))
    )
    i512 = np.ascontiguousarray(
        np.tile(np.arange(OC, dtype=np.float16)[None, :], (128, 1))
    )

    core_of = d_all >> 9
    in_maps = []
    for c in range(CORES):
        m = core_of == c
        s = s_all[m]
        dl = d_all[m] - c * DWIN

        # window-major slot packing: gene window w = s >> 7 gets WCH chunks
        # of 128 slots; pads get -1 (all-zero one-hots)
        sl_lin = np.full(NSLOT, -1.0, dtype=np.float32)
        dl_lin = np.full(NSLOT, -1.0, dtype=np.float32)
        o = np.argsort(s, kind="stable")
        s_o = s[o]
        dl_o = dl[o]
        wnd = s_o >> 7
        cnt = np.bincount(wnd, minlength=ST)
        assert cnt.max() <= WCH * 128, f"window overflow: {cnt.max()}"
        pos = 0
        for t in range(ST):
            n = int(cnt[t])
            base = t * WCH * 128
            sl_lin[base:base + n] = (s_o[pos:pos + n] - t * 128).astype(np.float32)
            dl_lin[base:base + n] = dl_o[pos:pos + n].astype(np.float32)
            pos += n

        sloc_t = np.ascontiguousarray(sl_lin.reshape(NCH, 128).T)
        dloc_t = np.ascontiguousarray(dl_lin.reshape(NCH, 128).T)

        xsT = np.ascontiguousarray(x[c * DWIN:(c + 1) * DWIN, :].T)

        in_maps.append(
            {
                "xT": xsT,
                "w": weight,
                "brep": brep,
                "i128": i128,
                "i512": i512,
                "sloc": sloc_t,
                "dloc": dloc_t,
            }
        )
    return in_maps


_NC = None


def _get_nc():
    global _NC
    if _NC is None:
        _NC = build_nc()
    return _NC


def kernel(x, weight, bias, edge_index, **run_kwargs):
    from concourse.bass_utils import run_bass_kernel_spmd

    nc = _get_nc()
    in_maps = make_in_maps(x, weight, bias, edge_index)
    res = run_bass_kernel_spmd(nc, in_maps, core_ids=list(range(CORES)), **run_kwargs)
    outs = res.results if hasattr(res, "results") else res
    full = np.empty((ND, OC), dtype=np.float32)
    for c in range(CORES):
        n = min(DWIN, ND - c * DWIN)
        full[c * DWIN:c * DWIN + n] = outs[c]["out"][:n]
    if run_kwargs:
        return full, res
    return full



# revision 5
# speedup vs baseline: 2.7770x; 2.7770x over previous
"""BipartiteGCN message-passing kernel for 8 TRN2 NeuronCores.

Math:  out = D_c^{-1/2} A^T D_r^{-1/2} (x @ W) + b
where A[s, d] = multiplicity of edge (gene s, drug d), s, d in [0, 4000).

Strategy (dst-window sharding, v2):
  - Core c owns drug (dst) window [512c, 512c+512).  Edges are sharded to
    cores by dst window and bucketed by (gene window 128, dst subwindow 128)
    (host-side layout only; all arithmetic happens on device).
  - Each core builds its dense count stripe A_c [4096 genes x 512 drugs] in
    SBUF with 128x128 one-hot x one-hot PE matmuls (fp16, 1 cycle/row), one
    [128,512] PSUM tile per gene window (4 dst-subwindow column groups x 3
    chunks).  One-hot builds alternate between DVE and GPSIMD engines.
  - xW is computed fully locally in float32r (1 cycle/row, no collective).
  - row_deg partials come free from the Act-engine PSUM->SBUF copy of A
    (accum_out); exchanged via a small AllGather (128KB) and summed locally.
    col_deg = ones^T @ A on the PE (local).
  - out = g * ((f*A)^T @ xW) + bias, GEMMs in bf16; g/bias fused into the
    output copy path.
"""

import sys

if "/opt/trn_rl_repo" not in sys.path:
    sys.path.insert(0, "/opt/trn_rl_repo")

import numpy as np

import concourse.bass as bass  # noqa: F401
import concourse.mybir as mybir
from concourse import bacc, tile

CORES = 8
DWIN = 512              # dst (drug) window per core
ND = 4000               # number of drugs
GD = 4096               # padded gene dim (src < 4000)
IC = 1024
OC = 512
GT = GD // 128          # 32 gene windows
DT = DWIN // 128        # 4 dst subwindows per core
CPB = 3                 # chunks per (gwin, dwin) bucket
NCH = GT * DT * CPB     # 384 chunks per core
NSLOT = NCH * 128       # 49152 edge slots per core

F32 = mybir.dt.float32
F32R = mybir.dt.float32r
F16 = mybir.dt.float16
BF16 = mybir.dt.bfloat16
AX = mybir.AxisListType
OP = mybir.AluOpType
ACT = mybir.ActivationFunctionType


def build_nc(debug_outputs=False):
    nc = bacc.Bacc(
        None,
        target_bir_lowering=False,
        debug=False,
        num_devices=CORES,
    )

    xT = nc.dram_tensor("xT", [IC, GD], F32R, kind="ExternalInput")
    w = nc.dram_tensor("w", [IC, OC], F32R, kind="ExternalInput")
    brep = nc.dram_tensor("brep", [128, OC], F32, kind="ExternalInput")
    i128 = nc.dram_tensor("i128", [128, 128], F16, kind="ExternalInput")
    sloc = nc.dram_tensor("sloc", [128, NCH], F32, kind="ExternalInput")
    dloc = nc.dram_tensor("dloc", [128, NCH], F32, kind="ExternalInput")
    out = nc.dram_tensor("out", [DWIN, OC], F32, kind="ExternalOutput")

    rdl = nc.dram_tensor("rdl", [128, GT], F32)            # rowdeg partial
    rds = nc.dram_tensor("rds", [CORES * 128, GT], F32, addr_space="Shared")
    cdl = nc.dram_tensor("cdl", [1, OC], F32)              # coldeg bounce

    Adbg = None
    if debug_outputs:
        Adbg = nc.dram_tensor("Adbg", [GD, OC], F32, kind="ExternalOutput")

    with tile.TileContext(nc) as tc:
        with (
            tc.tile_pool(name="const", bufs=1) as cpool,
            tc.tile_pool(name="work", bufs=3) as wpool,
            tc.tile_pool(name="oh", bufs=6) as ohpool,
            tc.tile_pool(name="apool", bufs=GT) as apool,
            tc.tile_pool(name="xwpool", bufs=GT) as xwpool,
            tc.tile_pool(name="psA", bufs=2, space="PSUM") as psA,
            tc.tile_pool(name="psB", bufs=4, space="PSUM") as psB,
            tc.tile_pool(name="psC", bufs=1, space="PSUM") as psC,
        ):
            # ---- constants ----
            ones_sb = cpool.tile([128, 1], BF16)
            nc.vector.memset(ones_sb[:], 1.0)
            i128_sb = cpool.tile([128, 128], F16)
            nc.sync.dma_start(i128_sb[:], i128[:])
            bias_sb = cpool.tile([128, OC], F32)
            nc.sync.dma_start(bias_sb[:], brep[:])
            sloc_sb = cpool.tile([128, NCH], F32)
            nc.sync.dma_start(sloc_sb[:], sloc[:])
            dloc_sb = cpool.tile([128, NCH], F32)
            nc.sync.dma_start(dloc_sb[:], dloc[:])
            # w tiles stay resident (f32r)
            w_sb = []
            for kt in range(8):
                wt = cpool.tile([128, OC], F32R, name=f"w{kt}")
                nc.sync.dma_start(wt[:], w[kt * 128:(kt + 1) * 128, :])
                w_sb.append(wt)

            a_sb = [None] * GT
            xw_sb = [None] * GT
            rd_sb = cpool.tile([128, GT], F32)
            pcd = psC.tile([1, OC], F32, tag="cd")

            # ---- A-build (gene window t) + full local xW, interleaved ----
            for t in range(GT):
                pa = psA.tile([128, OC], F32, tag="bld", name=f"pa{t}")
                for dwin in range(DT):
                    for i in range(CPB):
                        c = (t * DT + dwin) * CPB + i
                        loh = ohpool.tile([128, 128], F16, tag="loh",
                                          name=f"loh{c}")
                        roh = ohpool.tile([128, 128], F16, tag="roh",
                                          name=f"roh{c}")
                        # alternate one-hot builds between DVE and GPSIMD
                        eng_l = nc.vector if (c % 2 == 0) else nc.gpsimd
                        eng_r = nc.gpsimd if (c % 2 == 0) else nc.vector
                        eng_l.tensor_scalar(
                            out=loh[:], in0=i128_sb[:],
                            scalar1=sloc_sb[:, c:c + 1], scalar2=None,
                            op0=OP.is_equal,
                        )
                        eng_r.tensor_scalar(
                            out=roh[:], in0=i128_sb[:],
                            scalar1=dloc_sb[:, c:c + 1], scalar2=None,
                            op0=OP.is_equal,
                        )
                        nc.tensor.matmul(
                            pa[:, dwin * 128:(dwin + 1) * 128],
                            loh[:], roh[:],
                            start=(i == 0), stop=(i == CPB - 1),
                            skip_group_check=True,
                        )
                # PSUM -> SBUF copy (cast bf16) + free rowdeg partial
                a_t = apool.tile([128, OC], BF16, tag="A", name=f"a{t}")
                nc.scalar.activation(
                    a_t[:], pa[:], ACT.Copy,
                    accum_out=rd_sb[:, t:t + 1],
                )
                a_sb[t] = a_t
                if debug_outputs:
                    dbg = wpool.tile([128, OC], F32, tag="dbg", name=f"dbg{t}")
                    nc.vector.tensor_copy(dbg[:], a_t[:])
                    nc.sync.dma_start(Adbg[t * 128:(t + 1) * 128, :], dbg[:])
                # coldeg accumulation: pcd += ones^T @ A_t   (bf16, 1cyc)
                nc.tensor.matmul(
                    pcd[:], ones_sb[:], a_t[:],
                    start=(t == 0), stop=(t == GT - 1),
                )
                # interleave one xW block every 4 gene windows:
                # block i computes xw rows [512i, 512i+512)
                if t % 4 == 3:
                    i_b = t // 4
                    pb = [psB.tile([128, OC], F32, tag="bp", name=f"pb{i_b}_{j}")
                          for j in range(4)]
                    for kt in range(8):
                        xt_t = wpool.tile([128, OC], F32R, tag="xT",
                                          name=f"xt{i_b}_{kt}")
                        nc.sync.dma_start(
                            xt_t[:],
                            xT[kt * 128:(kt + 1) * 128,
                               i_b * 512:(i_b + 1) * 512],
                        )
                        for j in range(4):
                            nc.tensor.matmul(
                                pb[j][:],
                                xt_t[:, j * 128:(j + 1) * 128],
                                w_sb[kt][:],
                                start=(kt == 0), stop=(kt == 7),
                            )
                    for j in range(4):
                        xw_t = xwpool.tile([128, OC], BF16, tag="XW",
                                           name=f"xw{i_b}_{j}")
                        nc.scalar.activation(xw_t[:], pb[j][:], ACT.Copy)
                        xw_sb[4 * i_b + j] = xw_t

            # ---- rowdeg partial exchange (AllGather + local sum) ----
            nc.sync.dma_start(rdl[:], rd_sb[:])
            nc.gpsimd.collective_compute(
                "AllGather",
                OP.bypass,
                replica_groups=[list(range(CORES))],
                ins=[rdl[:].opt()],
                outs=[rds[:].opt()],
            )
            deg = cpool.tile([128, GT], F32)
            parts = []
            for r in range(CORES):
                pt = wpool.tile([128, GT], F32, tag="rdp", bufs=CORES,
                                name=f"rdp{r}")
                nc.sync.dma_start(pt[:], rds[r * 128:(r + 1) * 128, :])
                parts.append(pt)
            nc.vector.tensor_tensor(
                out=deg[:], in0=parts[0][:], in1=parts[1][:], op=OP.add)
            for r in range(2, CORES):
                nc.vector.tensor_tensor(
                    out=deg[:], in0=deg[:], in1=parts[r][:], op=OP.add)

            # f = (deg>0)/sqrt(max(deg,1))
            t1 = cpool.tile([128, GT], F32)
            nc.vector.tensor_scalar(
                out=t1[:], in0=deg[:], scalar1=1.0, scalar2=None, op0=OP.max)
            nc.scalar.sqrt(t1[:], t1[:])
            nc.vector.reciprocal(t1[:], t1[:])
            fmask = cpool.tile([128, GT], F32)
            nc.vector.tensor_scalar(
                out=fmask[:], in0=deg[:], scalar1=0.5, scalar2=None,
                op0=OP.is_gt)
            f_sb = cpool.tile([128, GT], F32)
            nc.vector.tensor_tensor(
                out=f_sb[:], in0=t1[:], in1=fmask[:], op=OP.mult)

            # ---- coldeg -> g  ([1,512] -> [128,4], drug dt*128+p on part p)
            cd_row = cpool.tile([1, OC], F32)
            nc.vector.tensor_copy(cd_row[:], pcd[:])
            nc.sync.dma_start(cdl[:], cd_row[:])
            cd_sb = cpool.tile([128, DT], F32)
            for kq in range(DT):
                nc.sync.dma_start(
                    cd_sb[:, kq:kq + 1], cdl[0:1, kq * 128:(kq + 1) * 128])
            g1 = cpool.tile([128, DT], F32)
            nc.vector.tensor_scalar(
                out=g1[:], in0=cd_sb[:], scalar1=1.0, scalar2=None, op0=OP.max)
            nc.scalar.sqrt(g1[:], g1[:])
            nc.vector.reciprocal(g1[:], g1[:])
            gmask = cpool.tile([128, DT], F32)
            nc.vector.tensor_scalar(
                out=gmask[:], in0=cd_sb[:], scalar1=0.5, scalar2=None,
                op0=OP.is_gt)
            g_sb = cpool.tile([128, DT], F32)
            nc.vector.tensor_tensor(
                out=g_sb[:], in0=g1[:], in1=gmask[:], op=OP.mult)

            # ---- F: out = (f*A)^T @ xW  (bf16, accumulated over windows)
            po = [psB.tile([128, OC], F32, tag="bp", name=f"po{j}")
                  for j in range(4)]
            for t in range(GT):
                nc.vector.tensor_scalar(
                    out=a_sb[t][:], in0=a_sb[t][:],
                    scalar1=f_sb[:, t:t + 1], scalar2=None, op0=OP.mult)
                for dt_i in range(4):
                    nc.tensor.matmul(
                        po[dt_i][:],
                        a_sb[t][:, dt_i * 128:(dt_i + 1) * 128],
                        xw_sb[t][:],
                        start=(t == 0), stop=(t == GT - 1),
                    )

            # ---- g-scale (Act copy) + bias (DVE) + store ----
            for dt_i in range(4):
                og = wpool.tile([128, OC], F32, tag="og", name=f"og{dt_i}")
                nc.scalar.activation(
                    og[:], po[dt_i][:], ACT.Copy,
                    scale=g_sb[:, dt_i:dt_i + 1],
                )
                nc.vector.tensor_tensor(
                    out=og[:], in0=og[:], in1=bias_sb[:], op=OP.add)
                nc.sync.dma_start(out[dt_i * 128:(dt_i + 1) * 128, :], og[:])

    nc.finalize()
    return nc


def make_in_maps(x, weight, bias, edge_index):
    """Host-side sharding/layout only: no arithmetic on tensor values."""
    x = np.asarray(x, dtype=np.float32)
    weight = np.ascontiguousarray(np.asarray(weight, dtype=np.float32))
    bias = np.asarray(bias, dtype=np.float32)
    ei = np.asarray(edge_index)
    s_all = ei[0].astype(np.int64)
    d_all = ei[1].astype(np.int64)
    assert s_all.min() >= 0 and s_all.max() < ND, "src ids out of range"
    assert d_all.min() >= 0 and d_all.max() < ND, "dst ids out of range"

    brep = np.ascontiguousarray(
        np.tile(bias[None, :], (128, 1)).astype(np.float32))
    i128 = np.ascontiguousarray(
        np.tile(np.arange(128, dtype=np.float16)[None, :], (128, 1)))

    x4 = np.zeros((GD, IC), dtype=np.float32)
    x4[:ND] = x[:ND]
    xT = np.ascontiguousarray(x4.T)

    core_of = d_all >> 9
    in_maps = []
    for c in range(CORES):
        m = core_of == c
        s = s_all[m]
        dl = d_all[m] - c * DWIN

        # bucket by (gene window, dst subwindow); CPB chunks per bucket
        b = (s >> 7) * DT + (dl >> 7)
        o = np.argsort(b, kind="stable")
        s_o, dl_o, b_o = s[o], dl[o], b[o]
        cnt = np.bincount(b_o, minlength=GT * DT)
        assert cnt.max() <= CPB * 128, f"bucket overflow: {cnt.max()}"

        sl_lin = np.full(NSLOT, -1.0, dtype=np.float32)
        dl_lin = np.full(NSLOT, -1.0, dtype=np.float32)
        pos = 0
        for bb in range(GT * DT):
            n = int(cnt[bb])
            base = bb * CPB * 128
            sl_lin[base:base + n] = (s_o[pos:pos + n] & 127).astype(np.float32)
            dl_lin[base:base + n] = (dl_o[pos:pos + n] & 127).astype(np.float32)
            pos += n

        sloc_t = np.ascontiguousarray(sl_lin.reshape(NCH, 128).T)
        dloc_t = np.ascontiguousarray(dl_lin.reshape(NCH, 128).T)

        in_maps.append(
            {
                "xT": xT,
                "w": weight,
                "brep": brep,
                "i128": i128,
                "sloc": sloc_t,
                "dloc": dloc_t,
            }
        )
    return in_maps


_NC = None


def _get_nc():
    global _NC
    if _NC is None:
        _NC = build_nc()
    return _NC


def kernel(x, weight, bias, edge_index, **run_kwargs):
    from concourse.bass_utils import run_bass_kernel_spmd

    nc = _get_nc()
    in_maps = make_in_maps(x, weight, bias, edge_index)
    res = run_bass_kernel_spmd(nc, in_maps, core_ids=list(range(CORES)),
                               **run_kwargs)
    outs = res.results if hasattr(res, "results") else res
    full = np.empty((ND, OC), dtype=np.float32)
    for c in range(CORES):
        n = min(DWIN, ND - c * DWIN)
        full[c * DWIN:c * DWIN + n] = outs[c]["out"][:n]
    if run_kwargs:
        return full, res
    return full


# revision 17
# speedup vs baseline: 3.8881x; 1.4001x over previous
"""BipartiteGCN message-passing kernel for 8 TRN2 NeuronCores.

Math:  out = D_c^{-1/2} A^T D_r^{-1/2} (x @ W) + b
where A[s, d] = multiplicity of edge (gene s, drug d), s, d in [0, 4000).

Strategy (dst-window sharding, v3 — RDMA exchanges):
  - Core c owns drug (dst) window [512c, 512c+512).  Edges are sharded to
    cores by dst window and bucketed by (gene window 128, dst subwindow 128)
    (host-side layout only; all arithmetic happens on device).
  - A stripe [4096 x 512] built in SBUF from 128x128 one-hot PE matmuls
    (fp16); one-hot builds alternate between DVE and GPSIMD.
  - xW is row-sharded: each core computes its 512-gene stripe in float32r,
    then remote_dma_broadcast's it (bf16, rank-offset landing slot) to all 8
    cores' SBUF — no collective, ~10us on the DMA engines.
  - row_deg partials (from the Act-engine A-copy accum_out) are exchanged the
    same way (128B/partition broadcast).  col_deg = ones^T @ A on the PE.
  - A kernel-start barrier + sem clears make the remote writes safe across
    dispatches.
  - out = g * ((f*A)^T @ xW) + bias, GEMMs in bf16; g/bias fused into the
    output copy path.
"""

import sys

if "/opt/trn_rl_repo" not in sys.path:
    sys.path.insert(0, "/opt/trn_rl_repo")

import numpy as np

import concourse.bass as bass  # noqa: F401
import concourse.libnrt as _libnrt
import concourse.mybir as mybir
from concourse import bacc, tile
from concourse.ap import AP as APcls

# The cost-model simulator resolves remote-DMA destinations through driver
# ioctls that this container's nrt shim cannot answer.  The device program
# itself only uses *relative* rdests (the Q7 ucode XORs at runtime), so an
# identity logical->physical mapping is a faithful model.  Only installed
# when the real ioctl fails.
try:
    _libnrt.get_trn2_nc_mapping()
except Exception:
    _libnrt.get_trn2_nc_mapping.cache_clear()
    _libnrt.get_trn2_nc_mapping = lambda: {(d, i): i for d in range(8)
                                           for i in range(8)}
try:
    _libnrt.get_device_id_to_routing_id_mapping()
except Exception:
    _libnrt.get_device_id_to_routing_id_mapping.cache_clear()
    _libnrt.get_device_id_to_routing_id_mapping = lambda: {d: d
                                                           for d in range(8)}
    import concourse.bass_interp as _bi

    _bi.get_device_id_to_routing_id_mapping = \
        _libnrt.get_device_id_to_routing_id_mapping

CORES = 8
DWIN = 512              # dst (drug) window per core
ND = 4000               # number of drugs
GD = 4096               # padded gene dim (src < 4000)
IC = 1024
OC = 512
GT = GD // 128          # 32 gene windows
DT = DWIN // 128        # 4 dst subwindows per core
CPB = 3                 # chunks per (gwin, dwin) bucket
NCH = GT * DT * CPB     # 384 chunks per core
NSLOT = NCH * 128       # 49152 edge slots per core

F32 = mybir.dt.float32
F32R = mybir.dt.float32r
F16 = mybir.dt.float16
BF16 = mybir.dt.bfloat16
OP = mybir.AluOpType
ACT = mybir.ActivationFunctionType

RG = [list(range(CORES))]
RDESTS = [(0, k) for k in range(CORES)]


def build_nc(debug_outputs=False):
    nc = bacc.Bacc(
        None,
        target_bir_lowering=False,
        debug=False,
        num_devices=CORES,
    )

    xT = nc.dram_tensor("xT", [IC, DWIN], F32R, kind="ExternalInput")
    w = nc.dram_tensor("w", [IC, OC], F32R, kind="ExternalInput")
    brep = nc.dram_tensor("brep", [128, OC], F32, kind="ExternalInput")
    i128 = nc.dram_tensor("i128", [128, 128], F16, kind="ExternalInput")
    sloc = nc.dram_tensor("sloc", [128, NCH], F32, kind="ExternalInput")
    dloc = nc.dram_tensor("dloc", [128, NCH], F32, kind="ExternalInput")
    out = nc.dram_tensor("out", [DWIN, OC], F32, kind="ExternalOutput")
    cdl = nc.dram_tensor("cdl", [1, OC], F32)              # coldeg bounce

    # sems / rank / wait-target registers, set up before the tile block so
    # the scheduler cannot reorder them after their readers
    xw_sem = nc.alloc_semaphore("xw_rsem")
    rd_sem = nc.alloc_semaphore("rd_rsem")
    lsem = nc.alloc_semaphore("rdma_lsem")
    nc.gpsimd.sem_clear(xw_sem)
    nc.gpsimd.sem_clear(rd_sem)
    nc.gpsimd.sem_clear(lsem)
    rank = nc.gpsimd.cc_rank(RG)
    rpe = nc.tensor.alloc_register("xw_tgt")
    nc.tensor.reg_mov(rpe, 2 * CORES)
    rdve = nc.vector.alloc_register("rd_tgt")
    nc.vector.reg_mov(rdve, 2 * CORES)
    rbar = nc.gpsimd.alloc_register("bar_tgt")
    nc.gpsimd.reg_mov(rbar, nc.bir_kernel_barrier_sem_inc)

    with tile.TileContext(nc) as tc:
        with (
            tc.tile_pool(name="const", bufs=1) as cpool,
            tc.tile_pool(name="work", bufs=3) as wpool,
            tc.tile_pool(name="oh", bufs=6) as ohpool,
            tc.tile_pool(name="apool", bufs=GT) as apool,
            tc.tile_pool(name="psA", bufs=2, space="PSUM") as psA,
            tc.tile_pool(name="psB", bufs=4, space="PSUM") as psB,
            tc.tile_pool(name="psC", bufs=1, space="PSUM") as psC,
        ):
            # ---- RDMA landing buffers / semaphores / rank ----
            xwfull = cpool.tile([128, GT * OC], BF16, name="xwfull")
            rdall = cpool.tile([128, CORES * GT], F32, name="rdall")
            xwstage = cpool.tile([128, 4 * OC], BF16, name="xwstage")

            def dyn_slice(t, width):
                ap0 = t[:, 0:width]
                return APcls(
                    tensor=ap0.tensor,
                    offset=ap0.offset + rank * width,
                    ap=ap0.ap,
                    dep_tracking_offset=ap0.offset,
                )

            # ---- constants ----
            ones_sb = cpool.tile([128, 1], BF16)
            nc.vector.memset(ones_sb[:], 1.0)
            i128_sb = cpool.tile([128, 128], F16)
            nc.sync.dma_start(i128_sb[:], i128[:])
            bias_sb = cpool.tile([128, OC], F32)
            nc.sync.dma_start(bias_sb[:], brep[:])
            sloc_sb = cpool.tile([128, NCH], F32)
            nc.sync.dma_start(sloc_sb[:], sloc[:])
            dloc_sb = cpool.tile([128, NCH], F32)
            nc.sync.dma_start(dloc_sb[:], dloc[:])
            w_sb = []
            for kt in range(8):
                wt = cpool.tile([128, OC], F32R, name=f"w{kt}")
                nc.sync.dma_start(wt[:], w[kt * 128:(kt + 1) * 128, :])
                w_sb.append(wt)

            # ---- B: local xW stripe (f32r) -> xwstage (bf16) ----
            pb = [psB.tile([128, OC], F32, tag="bp", name=f"pb{j}")
                  for j in range(4)]
            for kt in range(8):
                xt_t = wpool.tile([128, DWIN], F32R, tag="xT", name=f"xt{kt}")
                nc.sync.dma_start(xt_t[:], xT[kt * 128:(kt + 1) * 128, :])
                for j in range(4):
                    nc.tensor.matmul(
                        pb[j][:],
                        xt_t[:, j * 128:(j + 1) * 128],
                        w_sb[kt][:],
                        start=(kt == 0), stop=(kt == 7),
                    )
            for j in range(4):
                nc.scalar.activation(
                    xwstage[:, j * OC:(j + 1) * OC], pb[j][:], ACT.Copy)

            # prep the xw broadcast now; triggered mid A-build after barrier
            nc.gpsimd.remote_dma_broadcast(
                dyn_slice(xwfull, 4 * OC), xwstage[:],
                remote_sem=xw_sem, local_sem=lsem, rdests=RDESTS,
            )

            a_sb = [None] * GT
            rd_sb = cpool.tile([128, GT], F32)
            pcd = psC.tile([1, OC], F32, tag="cd")

            # ---- A-build (gene window t) ----
            for t in range(GT):
                pa = psA.tile([128, OC], F32, tag="bld", name=f"pa{t}")
                for dwin in range(DT):
                    for i in range(CPB):
                        c = (t * DT + dwin) * CPB + i
                        loh = ohpool.tile([128, 128], F16, tag="loh",
                                          name=f"loh{c}")
                        roh = ohpool.tile([128, 128], F16, tag="roh",
                                          name=f"roh{c}")
                        eng_l = nc.vector if (c % 2 == 0) else nc.gpsimd
                        eng_r = nc.gpsimd if (c % 2 == 0) else nc.vector
                        eng_l.tensor_scalar(
                            out=loh[:], in0=i128_sb[:],
                            scalar1=sloc_sb[:, c:c + 1], scalar2=None,
                            op0=OP.is_equal,
                        )
                        eng_r.tensor_scalar(
                            out=roh[:], in0=i128_sb[:],
                            scalar1=dloc_sb[:, c:c + 1], scalar2=None,
                            op0=OP.is_equal,
                        )
                        nc.tensor.matmul(
                            pa[:, dwin * 128:(dwin + 1) * 128],
                            loh[:], roh[:],
                            start=(i == 0), stop=(i == CPB - 1),
                            skip_group_check=True,
                        )
                a_t = apool.tile([128, OC], BF16, tag="A", name=f"a{t}")
                nc.scalar.activation(
                    a_t[:], pa[:], ACT.Copy,
                    accum_out=rd_sb[:, t:t + 1],
                )
                a_sb[t] = a_t
                nc.tensor.matmul(
                    pcd[:], ones_sb[:], a_t[:],
                    start=(t == 0), stop=(t == GT - 1),
                )
                if t == 7:
                    # fire xw once all cores entered the kernel (sems cleared);
                    # the barrier wait rides on the trigger itself so the
                    # scheduler cannot split them
                    nc._bir_kernel_barrier_sem_replica_groups.extend(
                        set(g) for g in RG)
                    assert nc._bir_kernel_barrier_sem is not None
                    nc.gpsimd.trigger_dma(count=1).wait_op(
                        nc._bir_kernel_barrier_sem, rbar, "sem-ge")

            # ---- rowdeg partial broadcast ----
            nc.gpsimd.remote_dma_broadcast(
                dyn_slice(rdall, GT), rd_sb[:],
                remote_sem=rd_sem, local_sem=lsem, rdests=RDESTS,
            )
            nc.gpsimd.trigger_dma(count=1)

            # ---- sum partials, f = (deg>0)/sqrt(max(deg,1)) ----
            deg = cpool.tile([128, GT], F32)
            nc.vector.tensor_tensor(
                out=deg[:], in0=rdall[:, 0:GT], in1=rdall[:, GT:2 * GT],
                op=OP.add).wait_op(rd_sem, rdve, "sem-ge")
            for r in range(2, CORES):
                nc.vector.tensor_tensor(
                    out=deg[:], in0=deg[:], in1=rdall[:, r * GT:(r + 1) * GT],
                    op=OP.add)
            t1 = cpool.tile([128, GT], F32)
            nc.vector.tensor_scalar(
                out=t1[:], in0=deg[:], scalar1=1.0, scalar2=None, op0=OP.max)
            nc.scalar.sqrt(t1[:], t1[:])
            nc.vector.reciprocal(t1[:], t1[:])
            fmask = cpool.tile([128, GT], F32)
            nc.vector.tensor_scalar(
                out=fmask[:], in0=deg[:], scalar1=0.5, scalar2=None,
                op0=OP.is_gt)
            f_sb = cpool.tile([128, GT], F32)
            nc.vector.tensor_tensor(
                out=f_sb[:], in0=t1[:], in1=fmask[:], op=OP.mult)

            # ---- coldeg -> g  ([1,512] -> [128,4]) ----
            cd_row = cpool.tile([1, OC], F32)
            nc.vector.tensor_copy(cd_row[:], pcd[:])
            nc.sync.dma_start(cdl[:], cd_row[:])
            cd_sb = cpool.tile([128, DT], F32)
            for kq in range(DT):
                nc.sync.dma_start(
                    cd_sb[:, kq:kq + 1], cdl[0:1, kq * 128:(kq + 1) * 128])
            g1 = cpool.tile([128, DT], F32)
            nc.vector.tensor_scalar(
                out=g1[:], in0=cd_sb[:], scalar1=1.0, scalar2=None, op0=OP.max)
            nc.scalar.sqrt(g1[:], g1[:])
            nc.vector.reciprocal(g1[:], g1[:])
            gmask = cpool.tile([128, DT], F32)
            nc.vector.tensor_scalar(
                out=gmask[:], in0=cd_sb[:], scalar1=0.5, scalar2=None,
                op0=OP.is_gt)
            g_sb = cpool.tile([128, DT], F32)
            nc.vector.tensor_tensor(
                out=g_sb[:], in0=g1[:], in1=gmask[:], op=OP.mult)

            # ---- F: out = (f*A)^T @ xW  (bf16) ----
            po = [psB.tile([128, OC], F32, tag="bp", name=f"po{j}")
                  for j in range(4)]
            for t in range(GT):
                nc.vector.tensor_scalar(
                    out=a_sb[t][:], in0=a_sb[t][:],
                    scalar1=f_sb[:, t:t + 1], scalar2=None, op0=OP.mult)
                for dt_i in range(4):
                    mm = nc.tensor.matmul(
                        po[dt_i][:],
                        a_sb[t][:, dt_i * 128:(dt_i + 1) * 128],
                        xwfull[:, t * OC:(t + 1) * OC],
                        start=(t == 0), stop=(t == GT - 1),
                    )
                    if t == 0:
                        # gate the psum accumulation chain on xw arrival
                        mm.wait_op(xw_sem, rpe, "sem-ge")

            # ---- g-scale (Act) + bias (DVE) + store ----
            for dt_i in range(4):
                og = wpool.tile([128, OC], F32, tag="og", name=f"og{dt_i}")
                nc.scalar.activation(
                    og[:], po[dt_i][:], ACT.Copy,
                    scale=g_sb[:, dt_i:dt_i + 1],
                )
                nc.vector.tensor_tensor(
                    out=og[:], in0=og[:], in1=bias_sb[:], op=OP.add)
                nc.sync.dma_start(out[dt_i * 128:(dt_i + 1) * 128, :], og[:])

    nc.finalize()
    return nc


def make_in_maps(x, weight, bias, edge_index):
    """Host-side sharding/layout only: no arithmetic on tensor values."""
    x = np.asarray(x, dtype=np.float32)
    weight = np.ascontiguousarray(np.asarray(weight, dtype=np.float32))
    bias = np.asarray(bias, dtype=np.float32)
    ei = np.asarray(edge_index)
    s_all = ei[0].astype(np.int64)
    d_all = ei[1].astype(np.int64)
    assert s_all.min() >= 0 and s_all.max() < ND, "src ids out of range"
    assert d_all.min() >= 0 and d_all.max() < ND, "dst ids out of range"

    brep = np.ascontiguousarray(
        np.tile(bias[None, :], (128, 1)).astype(np.float32))
    i128 = np.ascontiguousarray(
        np.tile(np.arange(128, dtype=np.float16)[None, :], (128, 1)))

    x4 = np.zeros((GD, IC), dtype=np.float32)
    x4[:ND] = x[:ND]

    core_of = d_all >> 9
    in_maps = []
    for c in range(CORES):
        m = core_of == c
        s = s_all[m]
        dl = d_all[m] - c * DWIN

        b = (s >> 7) * DT + (dl >> 7)
        o = np.argsort(b, kind="stable")
        s_o, dl_o = s[o], dl[o]
        cnt = np.bincount(b[o], minlength=GT * DT)
        assert cnt.max() <= CPB * 128, f"bucket overflow: {cnt.max()}"

        sl_lin = np.full(NSLOT, -1.0, dtype=np.float32)
        dl_lin = np.full(NSLOT, -1.0, dtype=np.float32)
        pos = 0
        for bb in range(GT * DT):
            n = int(cnt[bb])
            base = bb * CPB * 128
            sl_lin[base:base + n] = (s_o[pos:pos + n] & 127).astype(np.float32)
            dl_lin[base:base + n] = (dl_o[pos:pos + n] & 127).astype(np.float32)
            pos += n

        sloc_t = np.ascontiguousarray(sl_lin.reshape(NCH, 128).T)
        dloc_t = np.ascontiguousarray(dl_lin.reshape(NCH, 128).T)
        xTs = np.ascontiguousarray(x4[c * DWIN:(c + 1) * DWIN].T)

        in_maps.append(
            {
                "xT": xTs,
                "w": weight,
                "brep": brep,
                "i128": i128,
                "sloc": sloc_t,
                "dloc": dloc_t,
            }
        )
    return in_maps


_NC = None


def _get_nc():
    global _NC
    if _NC is None:
        _NC = build_nc()
    return _NC


def kernel(x, weight, bias, edge_index, **run_kwargs):
    from concourse.bass_utils import run_bass_kernel_spmd

    nc = _get_nc()
    in_maps = make_in_maps(x, weight, bias, edge_index)
    res = run_bass_kernel_spmd(nc, in_maps, core_ids=list(range(CORES)),
                               **run_kwargs)
    outs = res.results if hasattr(res, "results") else res
    full = np.empty((ND, OC), dtype=np.float32)
    for c in range(CORES):
        n = min(DWIN, ND - c * DWIN)
        full[c * DWIN:c * DWIN + n] = outs[c]["out"][:n]
    if run_kwargs:
        return full, res
    return full
